# revision 1
# baseline (speedup 1.0000x reference)
"""GraphSAGE 2-layer GNN + MLP head on 8 Trainium2 NeuronCores.

Strategy (dst-sharded, dense-adjacency scatter):
  - Shard destination nodes across the 8 cores (1250 nodes each).
  - The per-edge gather+transform+scatter-mean collapses algebraically:
        m_e = relu(x[src_e] @ W);  s[d] = sum_{e: dst_e=d} m_e
    =>  y = relu(x @ W) per node (10k rows, not 640k), then
        s = y^T-contraction with A_k, where A_k[src, dst] = edge
        multiplicity counts for this core's dst shard.
    A_k is built host-side (that is the edge->core sharding step) and
    stored in fp8_e4m3 (counts 1..3 are exact), resident in SBUF and
    reused by both GNN layers.
  - All feature-space matmuls run feature-major (features on partitions)
    so no on-device transposes are needed until the final logits.
  - Between layers: one fp8 AllGather of the per-shard activations y1.
  - L2-row-normalization uses a ones-vector matmul for the cross-partition
    reduction and a K=1 broadcast matmul for the per-node inverse norm.
"""

import numpy as np
import ml_dtypes

import concourse.bacc as bacc
import concourse.mybir as mybir
from concourse import tile
from concourse.bass_utils import run_bass_kernel_spmd

N_NODES = 10000
N_CORES = 8
SHARD = N_NODES // N_CORES  # 1250
F = 128                      # hidden/feature dim
FOUT = 64                    # output classes
P = 128                      # partitions
KC = (N_NODES + P - 1) // P  # 79 src chunks (last = 16 rows)
JC = (SHARD + P - 1) // P    # 10 shard node chunks (last = 98 rows)
NCHUNKS = [(0, 512), (512, 512), (1024, SHARD - 1024)]  # moving-dim chunks

FP8 = mybir.dt.float8e4
BF16 = mybir.dt.bfloat16
F32 = mybir.dt.float32

NP_FP8 = ml_dtypes.float8_e4m3
NP_BF16 = ml_dtypes.bfloat16


def _kc(k):
    return min(P, N_NODES - k * P)


def _jc(j):
    return min(P, SHARD - j * P)


def build():
    nc = bacc.Bacc("TRN2", target_bir_lowering=False, debug=False,
                   num_devices=N_CORES)

    # ---- external I/O (per-core data via in_maps) ----
    xt_d = nc.declare_dram_parameter("xt", [P, N_NODES], BF16, isOutput=False)
    xtsh_d = nc.declare_dram_parameter("xt_sh", [P, SHARD], BF16, isOutput=False)
    a8_d = nc.declare_dram_parameter("a8", [N_NODES, SHARD], FP8, isOutput=False)
    inv_d = nc.declare_dram_parameter("inv", [P, SHARD], F32, isOutput=False)
    lin_w0_d = nc.declare_dram_parameter("lin_w0", [F, F], BF16, isOutput=False)
    lin_w1_d = nc.declare_dram_parameter("lin_w1", [F, F], BF16, isOutput=False)
    agg_w0_d = nc.declare_dram_parameter("agg_w0", [2 * F, F], BF16, isOutput=False)
    agg_w1_d = nc.declare_dram_parameter("agg_w1", [2 * F, F], BF16, isOutput=False)
    mp_w1_d = nc.declare_dram_parameter("mp_w1", [F, F], BF16, isOutput=False)
    mp_w2_d = nc.declare_dram_parameter("mp_w2", [F, FOUT], BF16, isOutput=False)
    out_d = nc.declare_dram_parameter("out", [SHARD, FOUT], F32, isOutput=True)

    ident_d = nc.inline_tensor(np.eye(P, dtype=np.float32), name="ident")

    # internal DRAM for the inter-layer AllGather
    y1sh_d = nc.dram_tensor("y1sh_d", [SHARD, F], FP8)
    y1all_d = nc.dram_tensor("y1all_d", [N_NODES, F], FP8, addr_space="Shared")

    with tile.TileContext(nc) as tc:
        with (
            tc.tile_pool(name="persist", bufs=1) as pp,
            tc.tile_pool(name="work", bufs=2) as wp,
            tc.tile_pool(name="stage", bufs=3) as sp,
            tc.tile_pool(name="ps_s", bufs=2, space="PSUM") as ps_s,
            tc.tile_pool(name="ps_y", bufs=2, space="PSUM") as ps_y,
            tc.tile_pool(name="ps_big", bufs=2, space="PSUM") as ps_big,
            tc.tile_pool(name="ps_n2", bufs=1, space="PSUM") as ps_n2,
            tc.tile_pool(name="ps_z2", bufs=1, space="PSUM") as ps_z2,
        ):
            # ---- persistent SBUF ----
            a_sb = pp.tile([P, KC * SHARD], FP8)
            xt_sb = pp.tile([P, N_NODES], BF16)
            xtsh_sb = pp.tile([P, SHARD], BF16)
            inv_sb = pp.tile([P, SHARD], F32)
            y_sb = pp.tile([P, KC * F], FP8)
            lin_w0_sb = pp.tile([F, F], BF16)
            lin_w1_sb = pp.tile([F, F], BF16)
            aggw0t_sb = pp.tile([F, F], BF16)
            aggw0b_sb = pp.tile([F, F], BF16)
            aggw1t_sb = pp.tile([F, F], BF16)
            aggw1b_sb = pp.tile([F, F], BF16)
            mp_w1_sb = pp.tile([F, F], BF16)
            mp_w2_sb = pp.tile([F, FOUT], BF16)
            ident_sb = pp.tile([P, P], F32)
            ones_col = pp.tile([P, 1], BF16)
            ones_row = pp.tile([1, P], F32)

            # weights/x first so y0 can start ASAP; xt split so chunks land early
            nc.sync.dma_start(lin_w0_sb[:], lin_w0_d[:])
            for q in range(4):
                qw = N_NODES // 4
                nc.sync.dma_start(xt_sb[:, q * qw:(q + 1) * qw],
                                  xt_d[:, q * qw:(q + 1) * qw])
            nc.sync.dma_start(xtsh_sb[:], xtsh_d[:])
            nc.sync.dma_start(inv_sb[:], inv_d[:])
            nc.sync.dma_start(lin_w1_sb[:], lin_w1_d[:])
            nc.sync.dma_start(aggw0t_sb[:], agg_w0_d[0:F, :])
            nc.sync.dma_start(aggw0b_sb[:], agg_w0_d[F:2 * F, :])
            nc.sync.dma_start(aggw1t_sb[:], agg_w1_d[0:F, :])
            nc.sync.dma_start(aggw1b_sb[:], agg_w1_d[F:2 * F, :])
            nc.sync.dma_start(mp_w1_sb[:], mp_w1_d[:])
            nc.sync.dma_start(mp_w2_sb[:], mp_w2_d[:])
            nc.sync.dma_start(ident_sb[:], ident_d[:])
            nc.gpsimd.memset(ones_col[:], 1.0)
            nc.gpsimd.memset(ones_row[:], 1.0)

            # adjacency slab: chunk k of src rows at free offset k*SHARD
            for k in range(KC):
                kc = _kc(k)
                nc.sync.dma_start(
                    a_sb[0:kc, k * SHARD:(k + 1) * SHARD],
                    a8_d[k * P:k * P + kc, :],
                )

            # ---- y0 = relu(x @ lin_w0), node-major, fp8 ----
            for k in range(KC):
                kc = _kc(k)
                ps = ps_y.tile([P, F], F32, tag="ps_y")
                nc.tensor.matmul(ps[0:kc, :], xt_sb[:, k * P:k * P + kc],
                                 lin_w0_sb[:], start=True, stop=True)
                nc.scalar.activation(y_sb[0:kc, k * F:(k + 1) * F], ps[0:kc, :],
                                     mybir.ActivationFunctionType.Relu)

            def sage_layer(aggwt_sb, aggwb_sb, x_rhs_sb):
                """scatter-mean + concat-linear + relu + L2norm.
                Returns the normalized output, feature-major bf16 [P, SHARD]."""
                aggrT = wp.tile([P, SHARD], BF16, tag="aggrT")
                for n0, ns in NCHUNKS:
                    ps = ps_s.tile([P, 512], F32, tag="ps_s")
                    for k in range(KC):
                        kc = _kc(k)
                        nc.tensor.matmul(
                            ps[:, 0:ns],
                            y_sb[0:kc, k * F:(k + 1) * F],
                            a_sb[0:kc, k * SHARD + n0:k * SHARD + n0 + ns],
                            start=(k == 0), stop=(k == KC - 1),
                        )
                    nc.vector.tensor_tensor(aggrT[:, n0:n0 + ns], ps[:, 0:ns],
                                            inv_sb[:, n0:n0 + ns],
                                            mybir.AluOpType.mult)

                hT = wp.tile([P, SHARD], F32, tag="hT")
                h2T = wp.tile([P, SHARD], BF16, tag="h2T")
                for n0, ns in NCHUNKS:
                    ps = ps_big.tile([P, 512], F32, tag="ps_big")
                    nc.tensor.matmul(ps[:, 0:ns], aggwt_sb[:],
                                     x_rhs_sb[:, n0:n0 + ns],
                                     start=True, stop=False)
                    nc.tensor.matmul(ps[:, 0:ns], aggwb_sb[:],
                                     aggrT[:, n0:n0 + ns],
                                     start=False, stop=True)
                    nc.scalar.activation(hT[:, n0:n0 + ns], ps[:, 0:ns],
                                         mybir.ActivationFunctionType.Relu)
                    nc.scalar.activation(h2T[:, n0:n0 + ns], hT[:, n0:n0 + ns],
                                         mybir.ActivationFunctionType.Square)

                nrm = wp.tile([1, SHARD], F32, tag="nrm")
                invn = wp.tile([1, SHARD], F32, tag="invn")
                for n0, ns in NCHUNKS:
                    psn = ps_n2.tile([1, 512], F32, tag="ps_n2")
                    nc.tensor.matmul(psn[:, 0:ns], ones_col[:],
                                     h2T[:, n0:n0 + ns], start=True, stop=True)
                    nc.scalar.activation(nrm[0:1, n0:n0 + ns], psn[:, 0:ns],
                                         mybir.ActivationFunctionType.Sqrt)
                nc.vector.tensor_scalar_max(nrm[:], nrm[:], 1e-12)
                nc.vector.reciprocal(invn[:], nrm[:])

                xoT = wp.tile([P, SHARD], BF16, tag="xoT")
                for n0, ns in NCHUNKS:
                    psb = ps_big.tile([P, 512], F32, tag="ps_big")
                    nc.tensor.matmul(psb[:, 0:ns], ones_row[:],
                                     invn[0:1, n0:n0 + ns], start=True, stop=True)
                    nc.vector.tensor_tensor(xoT[:, n0:n0 + ns], hT[:, n0:n0 + ns],
                                            psb[:, 0:ns], mybir.AluOpType.mult)
                return xoT

            # ---- layer 0 ----
            x1T = sage_layer(aggw0t_sb, aggw0b_sb, xtsh_sb)

            # ---- y1 = relu(x1 @ lin_w1) on our shard, allgather to all ----
            y1loc = wp.tile([P, JC * F], FP8, tag="y1loc")
            for j in range(JC):
                jc = _jc(j)
                ps = ps_y.tile([P, F], F32, tag="ps_y")
                nc.tensor.matmul(ps[0:jc, :], x1T[:, j * P:j * P + jc],
                                 lin_w1_sb[:], start=True, stop=True)
                nc.scalar.activation(y1loc[0:jc, j * F:(j + 1) * F], ps[0:jc, :],
                                     mybir.ActivationFunctionType.Relu)
                nc.sync.dma_start(y1sh_d[j * P:j * P + jc, :],
                                  y1loc[0:jc, j * F:(j + 1) * F])

            nc.gpsimd.collective_compute(
                "AllGather", mybir.AluOpType.bypass,
                replica_groups=[list(range(N_CORES))],
                ins=[y1sh_d[:]], outs=[y1all_d[:]],
            )
            for k in range(KC):
                kc = _kc(k)
                nc.sync.dma_start(y_sb[0:kc, k * F:(k + 1) * F],
                                  y1all_d[k * P:k * P + kc, :])

            # ---- layer 1 ----
            x2T = sage_layer(aggw1t_sb, aggw1b_sb, x1T)

            # ---- post_mp: (x2 @ mp_w1 + b1) @ mp_w2 + b2 (biases are zero) ----
            z1T = wp.tile([P, SHARD], BF16, tag="z1T")
            z2T = wp.tile([FOUT, SHARD], F32, tag="z2T")
            for n0, ns in NCHUNKS:
                ps1 = ps_big.tile([P, 512], F32, tag="ps_big")
                nc.tensor.matmul(ps1[:, 0:ns], mp_w1_sb[:], x2T[:, n0:n0 + ns],
                                 start=True, stop=True)
                nc.scalar.activation(z1T[:, n0:n0 + ns], ps1[:, 0:ns],
                                     mybir.ActivationFunctionType.Copy)
                ps2 = ps_z2.tile([FOUT, 512], F32, tag="ps_z2")
                nc.tensor.matmul(ps2[:, 0:ns], mp_w2_sb[:], z1T[:, n0:n0 + ns],
                                 start=True, stop=True)
                nc.scalar.activation(z2T[:, n0:n0 + ns], ps2[:, 0:ns],
                                     mybir.ActivationFunctionType.Copy)

            # ---- transpose logits to node-major + log_softmax over classes ----
            for j in range(JC):
                jc = _jc(j)
                pst = ps_y.tile([P, F], F32, tag="ps_y")
                nc.tensor.transpose(pst[0:jc, 0:FOUT],
                                    z2T[0:FOUT, j * P:j * P + jc],
                                    ident_sb[0:FOUT, 0:FOUT])
                rmax = sp.tile([P, 1], F32, tag="rmax")
                negmax = sp.tile([P, 1], F32, tag="negmax")
                nc.vector.tensor_reduce(rmax[0:jc, :], pst[0:jc, 0:FOUT],
                                        mybir.AxisListType.X, mybir.AluOpType.max)
                nc.vector.tensor_scalar_mul(negmax[0:jc, :], rmax[0:jc, :], -1.0)
                expt = sp.tile([P, FOUT], F32, tag="expt")
                sumexp = sp.tile([P, 1], F32, tag="sumexp")
                lnsum = sp.tile([P, 1], F32, tag="lnsum")
                adj = sp.tile([P, 1], F32, tag="adj")
                nc.scalar.activation(expt[0:jc, :], pst[0:jc, 0:FOUT],
                                     mybir.ActivationFunctionType.Exp,
                                     bias=negmax[0:jc, :],
                                     accum_out=sumexp[0:jc, :])
                nc.scalar.activation(lnsum[0:jc, :], sumexp[0:jc, :],
                                     mybir.ActivationFunctionType.Ln)
                nc.vector.tensor_sub(adj[0:jc, :], negmax[0:jc, :], lnsum[0:jc, :])
                outt = sp.tile([P, FOUT], F32, tag="outt")
                nc.vector.tensor_scalar_add(outt[0:jc, :], pst[0:jc, 0:FOUT],
                                            adj[0:jc, :])
                nc.sync.dma_start(out_d[j * P:j * P + jc, :], outt[0:jc, :])

    nc.compile()
    return nc


_NC = None


def _get_nc():
    global _NC
    if _NC is None:
        _NC = build()
    return _NC


def make_in_maps(inputs):
    x = np.asarray(inputs["x"], dtype=np.float32)
    ei = np.asarray(inputs["edge_index"])
    src = ei[0].astype(np.int64)
    dst = ei[1].astype(np.int64)

    cnt = np.bincount(dst, minlength=N_NODES).astype(np.float32)
    inv = (1.0 / np.maximum(cnt, 1.0)).astype(np.float32)

    # dense edge-count matrix [src, dst]
    flat = src * N_NODES + dst
    counts = np.bincount(flat, minlength=N_NODES * N_NODES)
    A = counts.reshape(N_NODES, N_NODES).astype(np.float32)
    A = np.clip(A, 0.0, 240.0)

    xt = np.ascontiguousarray(x.T).astype(NP_BF16)  # [128, 10000]

    def w(name):
        return np.ascontiguousarray(np.asarray(inputs[name], np.float32)).astype(NP_BF16)

    common = {
        "xt": xt,
        "lin_w0": w("lin_w0"), "lin_w1": w("lin_w1"),
        "agg_w0": w("agg_w0"), "agg_w1": w("agg_w1"),
        "mp_w1": w("mp_w1"), "mp_w2": w("mp_w2"),
    }
    in_maps = []
    for c in range(N_CORES):
        lo, hi = c * SHARD, (c + 1) * SHARD
        a_k = np.ascontiguousarray(A[:, lo:hi]).astype(NP_FP8)
        inv_k = np.ascontiguousarray(
            np.broadcast_to(inv[lo:hi][None, :], (P, SHARD))).astype(np.float32)
        in_maps.append({
            **common,
            "xt_sh": np.ascontiguousarray(xt[:, lo:hi]),
            "a8": a_k,
            "inv": inv_k,
        })
    return in_maps


def run(inputs, trace=False, **kwargs):
    nc = _get_nc()
    in_maps = make_in_maps(inputs)
    res = run_bass_kernel_spmd(nc, in_maps, core_ids=list(range(N_CORES)),
                               trace=trace, **kwargs)
    out = np.concatenate([res.results[c]["out"] for c in range(N_CORES)], axis=0)
    return out.astype(np.float32), res


def kernel(**inputs):
    out, _ = run(inputs, trace=False)
    return out



# revision 9
# speedup vs baseline: 1.6420x; 1.6420x over previous
"""GraphSAGE 2-layer GNN + MLP head on 8 Trainium2 NeuronCores (v2).

Strategy (dst-sharded, dense-adjacency scatter, fp8 DoubleRow):
  - Destination nodes sharded across 8 cores; node index space padded to
    1280 slots/core (10240 global slots = 80 full 128-chunks) so every
    matmul chunk is full and fp8 DoubleRow pairs align.
  - Scatter-mean collapses to  aggr = (relu(X W))^T A  with A[src,dst] the
    edge-count matrix in fp8 e4m3, resident in SBUF for both layers.
    Scatter matmuls run in fp8 DoubleRow (K=256 per instruction).
  - A is stored partition-major in DRAM ([128, 80, 1250]) so the whole
    12.8 MB loads with 10 dma_starts of 128x12.5KB descriptors.
  - Inter-layer: y1 AllGather split in two halves, pipelined against the
    layer-1 scatter of the first half.
  - Row L2-norm: ones-matmul partition reduction (broadcast to all 128
    partitions), scalar Sqrt, vector divide - all partition-parallel.
  - log_softmax: second post_mp matmul emits node-major logits directly
    (no transposes); exp/ln batched to avoid act-table thrash.
"""

import numpy as np
import ml_dtypes

import concourse.bacc as bacc
import concourse.mybir as mybir
from concourse import tile
from concourse.bass_utils import run_bass_kernel_spmd

N_NODES = 10000
N_CORES = 8
SHARD = N_NODES // N_CORES   # 1250 real dst nodes per core
P = 128
JC = 10                      # local 128-chunks per core (1280 slots)
SLOTS = JC * P               # 1280 padded slots per core
G = N_CORES * SLOTS          # 10240 padded global slots
KC = G // P                  # 80 src chunks
KP = KC // 2                 # 40 DoubleRow pairs
F = 128
FOUT = 64
NCHUNKS = [(0, 512), (512, 512), (1024, SHARD - 1024)]
JA = 6                       # allgather half A: local chunks 0..5 (pair-aligned)
JB = JC - JA                 # half B: chunks 6..9

FP8 = mybir.dt.float8e4
BF16 = mybir.dt.bfloat16
F32 = mybir.dt.float32
DR = mybir.MatmulPerfMode.DoubleRow
AF = mybir.ActivationFunctionType

NP_FP8 = ml_dtypes.float8_e4m3
NP_BF16 = ml_dtypes.bfloat16


def _jc(j):
    """real node count in local chunk j (last chunk is partial: 98)."""
    return min(P, SHARD - j * P)


def build():
    nc = bacc.Bacc("TRN2", target_bir_lowering=False, debug=False,
                   num_devices=N_CORES)

    # ---- external I/O ----
    xt_d = nc.declare_dram_parameter("xt", [P, G], BF16, isOutput=False)
    xtsh_d = nc.declare_dram_parameter("xt_sh", [P, SLOTS], BF16, isOutput=False)
    a8_d = nc.declare_dram_parameter("a8", [P, KC, SHARD], FP8, isOutput=False)
    inv_d = nc.declare_dram_parameter("inv", [P, SHARD], F32, isOutput=False)
    lin_w0_d = nc.declare_dram_parameter("lin_w0", [F, F], BF16, isOutput=False)
    lin_w1_d = nc.declare_dram_parameter("lin_w1", [F, F], BF16, isOutput=False)
    agg_w0_d = nc.declare_dram_parameter("agg_w0", [2 * F, F], BF16, isOutput=False)
    agg_w1_d = nc.declare_dram_parameter("agg_w1", [2 * F, F], BF16, isOutput=False)
    mp_w1_d = nc.declare_dram_parameter("mp_w1", [F, F], BF16, isOutput=False)
    mp_w2_d = nc.declare_dram_parameter("mp_w2", [F, FOUT], BF16, isOutput=False)
    out_d = nc.declare_dram_parameter("out", [SHARD, FOUT], F32, isOutput=True)

    # internal DRAM for the split inter-layer AllGather
    y1sh_a_d = nc.dram_tensor("y1sh_a_d", [P, JA * F], FP8)
    y1sh_b_d = nc.dram_tensor("y1sh_b_d", [P, JB * F], FP8)
    y1all_a_d = nc.dram_tensor("y1all_a_d", [N_CORES, P, JA * F], FP8,
                               addr_space="Shared")
    y1all_b_d = nc.dram_tensor("y1all_b_d", [N_CORES, P, JB * F], FP8,
                               addr_space="Shared")

    with tile.TileContext(nc) as tc:
        with (
            tc.tile_pool(name="persist", bufs=1) as pp,
            tc.tile_pool(name="work", bufs=2) as wp,
            tc.tile_pool(name="stage", bufs=2) as sp,
            tc.tile_pool(name="ps_s", bufs=1, space="PSUM") as ps_s,
            tc.tile_pool(name="ps_h", bufs=2, space="PSUM") as ps_h,
            tc.tile_pool(name="ps_b", bufs=1, space="PSUM") as ps_b,
            tc.tile_pool(name="ps_y", bufs=2, space="PSUM") as ps_y,
        ):
            # ---- persistent SBUF ----
            a_sb = pp.tile([P, KC, SHARD], FP8)
            xt_sb = pp.tile([P, G], BF16)
            xtsh_sb = pp.tile([P, SLOTS], BF16)
            inv_sb = pp.tile([P, SHARD], F32)
            y_sb = pp.tile([P, KC, F], FP8)
            y1loc = pp.tile([P, JC, F], FP8)
            x1T = pp.tile([P, SHARD], BF16)
            x2T = pp.tile([P, SHARD], BF16)
            z1T = pp.tile([P, SHARD], BF16)
            z2sb = pp.tile([P, JC * FOUT], F32)
            outsb = pp.tile([P, JC * FOUT], F32)
            rmax = pp.tile([P, JC], F32)
            negmax = pp.tile([P, JC], F32)
            sumexp = pp.tile([P, JC], F32)
            lnsum = pp.tile([P, JC], F32)
            adj = pp.tile([P, JC], F32)
            lin_w0_sb = pp.tile([F, F], BF16)
            lin_w1_sb = pp.tile([F, F], BF16)
            aggw0t_sb = pp.tile([F, F], BF16)
            aggw0b_sb = pp.tile([F, F], BF16)
            aggw1t_sb = pp.tile([F, F], BF16)
            aggw1b_sb = pp.tile([F, F], BF16)
            mp_w1_sb = pp.tile([F, F], BF16)
            mp_w2_sb = pp.tile([F, FOUT], BF16)
            ones_mat = pp.tile([P, P], BF16)
            eps_sb = pp.tile([P, 1], F32)

            # ---- front loads: y0 dependencies first, then the a8 stream ----
            nc.sync.dma_start(lin_w0_sb[:], lin_w0_d[:])
            XQ = G // 4
            for q in range(4):
                nc.sync.dma_start(xt_sb[:, q * XQ:(q + 1) * XQ],
                                  xt_d[:, q * XQ:(q + 1) * XQ])
            nc.sync.dma_start(xtsh_sb[:], xtsh_d[:])
            nc.sync.dma_start(inv_sb[:], inv_d[:])
            nc.sync.dma_start(lin_w1_sb[:], lin_w1_d[:])
            nc.sync.dma_start(aggw0t_sb[:], agg_w0_d[0:F, :])
            nc.sync.dma_start(aggw0b_sb[:], agg_w0_d[F:2 * F, :])
            nc.sync.dma_start(aggw1t_sb[:], agg_w1_d[0:F, :])
            nc.sync.dma_start(aggw1b_sb[:], agg_w1_d[F:2 * F, :])
            nc.sync.dma_start(mp_w1_sb[:], mp_w1_d[:])
            nc.sync.dma_start(mp_w2_sb[:], mp_w2_d[:])
            nc.gpsimd.memset(ones_mat[:], 1.0)
            nc.gpsimd.memset(eps_sb[:], 1e-24)
            nc.gpsimd.memset(y1loc[:, JC - 1, :], 0.0)
            nc.gpsimd.memset(rmax[:], 0.0)
            nc.gpsimd.memset(sumexp[:], 1.0)

            # adjacency: 10 slabs of 8 chunks, 12.5KB/partition descriptors
            ASLAB = 8
            for s in range(KC // ASLAB):
                nc.sync.dma_start(a_sb[:, s * ASLAB:(s + 1) * ASLAB, :],
                                  a8_d[:, s * ASLAB:(s + 1) * ASLAB, :])

            # ---- y0 = relu(x @ lin_w0), node-major fp8 (padded rows -> 0) ----
            for k in range(KC):
                ps = ps_y.tile([P, F], F32, tag="ps_y")
                nc.tensor.matmul(ps[:], xt_sb[:, k * P:(k + 1) * P],
                                 lin_w0_sb[:], start=True, stop=True)
                nc.scalar.activation(y_sb[:, k, :], ps[:], AF.Relu)

            def scatter(ps_list, kps, first, last):
                """fp8 DoubleRow scatter matmuls: 3 psum banks accumulate
                aggr^T = y^T A for the n-chunks; kp-outer for DMA pacing."""
                for kp in kps:
                    for i, (n0, ns) in enumerate(NCHUNKS):
                        nc.tensor.matmul(
                            ps_list[i][:, 0:ns],
                            y_sb[:, 2 * kp:2 * kp + 2, :],
                            a_sb[:, 2 * kp:2 * kp + 2, n0:n0 + ns],
                            start=(kp == first), stop=(kp == last),
                            perf_mode=DR,
                        )

            def sage_update(ps_list, aggwt_sb, aggwb_sb, xout):
                """aggr scale + concat-linear + relu + L2 row norm.
                Writes the normalized layer output into xout [P, SHARD] bf16."""
                for i, (n0, ns) in enumerate(NCHUNKS):
                    ps = ps_list[i]
                    aggrT = wp.tile([P, 512], BF16, tag="aggrT")
                    nc.vector.tensor_tensor(aggrT[:, 0:ns], ps[:, 0:ns],
                                            inv_sb[:, n0:n0 + ns],
                                            mybir.AluOpType.mult)
                    ph = ps_h.tile([P, 512], F32, tag="ph")
                    nc.tensor.matmul(ph[:, 0:ns], aggwt_sb[:],
                                     xtsh_sb[:, n0:n0 + ns] if xout is x1T
                                     else x1T[:, n0:n0 + ns],
                                     start=True, stop=False)
                    nc.tensor.matmul(ph[:, 0:ns], aggwb_sb[:],
                                     aggrT[:, 0:ns], start=False, stop=True)
                    hT = wp.tile([P, 512], F32, tag="hT")
                    nc.vector.tensor_scalar_max(hT[:, 0:ns], ph[:, 0:ns], 0.0)
                    h2 = wp.tile([P, 512], BF16, tag="h2")
                    nc.vector.tensor_tensor(h2[:, 0:ns], hT[:, 0:ns],
                                            hT[:, 0:ns], mybir.AluOpType.mult)
                    pb = ps_b.tile([P, 512], F32, tag="pb")
                    nc.tensor.matmul(pb[:, 0:ns], ones_mat[:], h2[:, 0:ns],
                                     start=True, stop=True)
                    nrm = wp.tile([P, 512], F32, tag="nrm")
                    nc.scalar.activation(nrm[:, 0:ns], pb[:, 0:ns], AF.Sqrt,
                                         bias=eps_sb[:])
                    rinv = wp.tile([P, 512], F32, tag="rinv")
                    nc.vector.reciprocal_approx_fast(rinv[:, 0:ns],
                                                     nrm[:, 0:ns])
                    nc.vector.tensor_tensor(xout[:, n0:n0 + ns], hT[:, 0:ns],
                                            rinv[:, 0:ns],
                                            mybir.AluOpType.mult)

            # ---- layer 0 ----
            ps_l0 = [ps_s.tile([P, 512], F32, tag=f"s{i}", name=f"ps_l0_{i}")
                     for i in range(3)]
            scatter(ps_l0, range(KP), 0, KP - 1)
            sage_update(ps_l0, aggw0t_sb, aggw0b_sb, x1T)

            # ---- y1 = relu(x1 @ lin_w1) on our shard; split AllGather ----
            def y1_chunks(j0, j1):
                for j in range(j0, j1):
                    jc = _jc(j)
                    ps = ps_y.tile([P, F], F32, tag="ps_y")
                    nc.tensor.matmul(ps[0:jc, :], x1T[:, j * P:j * P + jc],
                                     lin_w1_sb[:], start=True, stop=True)
                    nc.scalar.activation(y1loc[0:jc, j, :], ps[0:jc, :],
                                         AF.Relu)

            y1_chunks(0, JA)
            nc.sync.dma_start(y1sh_a_d[:], y1loc[:, 0:JA, :])
            nc.gpsimd.collective_compute(
                "AllGather", mybir.AluOpType.bypass,
                replica_groups=[list(range(N_CORES))],
                ins=[y1sh_a_d[:]], outs=[y1all_a_d[:]],
            )
            y1_chunks(JA, JC)
            nc.sync.dma_start(y1sh_b_d[:], y1loc[:, JA:JC, :])
            nc.gpsimd.collective_compute(
                "AllGather", mybir.AluOpType.bypass,
                replica_groups=[list(range(N_CORES))],
                ins=[y1sh_b_d[:]], outs=[y1all_b_d[:]],
            )
            # reload gathered y1 into y_sb (chunk k = c*JC + j)
            ysb_v = y_sb[:, :, :].rearrange("p (c j) f -> p c (j f)", c=N_CORES)
            nc.sync.dma_start(ysb_v[:, :, 0:JA * F],
                              y1all_a_d[:].transpose([1, 0, 2]))
            nc.sync.dma_start(ysb_v[:, :, JA * F:JC * F],
                              y1all_b_d[:].transpose([1, 0, 2]))

            # ---- layer 1: scatter half A first (overlaps AllGather B) ----
            kps_a = [c * (JC // 2) + q for c in range(N_CORES)
                     for q in range(JA // 2)]
            kps_b = [c * (JC // 2) + q for c in range(N_CORES)
                     for q in range(JA // 2, JC // 2)]
            ps_l1 = [ps_s.tile([P, 512], F32, tag=f"s{i}", name=f"ps_l1_{i}")
                     for i in range(3)]
            scatter(ps_l1, kps_a, kps_a[0], -1)
            scatter(ps_l1, kps_b, -1, kps_b[-1])
            sage_update(ps_l1, aggw1t_sb, aggw1b_sb, x2T)

            # ---- post_mp: z2 = (x2 @ mp_w1) @ mp_w2, node-major logits ----
            for i, (n0, ns) in enumerate(NCHUNKS):
                p1 = ps_h.tile([P, 512], F32, tag="ph")
                nc.tensor.matmul(p1[:, 0:ns], mp_w1_sb[:], x2T[:, n0:n0 + ns],
                                 start=True, stop=True)
                nc.scalar.activation(z1T[:, n0:n0 + ns], p1[:, 0:ns], AF.Copy)
            for j in range(JC):
                jc = _jc(j)
                p2 = ps_y.tile([P, F], F32, tag="ps_y")
                nc.tensor.matmul(p2[0:jc, 0:FOUT], z1T[:, j * P:j * P + jc],
                                 mp_w2_sb[:], start=True, stop=True)
                nc.scalar.activation(z2sb[0:jc, j * FOUT:(j + 1) * FOUT],
                                     p2[0:jc, 0:FOUT], AF.Copy)
                nc.vector.tensor_reduce(rmax[0:jc, j:j + 1],
                                        z2sb[0:jc, j * FOUT:(j + 1) * FOUT],
                                        mybir.AxisListType.X,
                                        mybir.AluOpType.max)

            # ---- log_softmax over classes, batched ----
            nc.vector.tensor_scalar_mul(negmax[:], rmax[:], -1.0)
            for j in range(JC):
                jc = _jc(j)
                expt = sp.tile([P, FOUT], F32, tag="expt")
                nc.scalar.activation(expt[0:jc, :],
                                     z2sb[0:jc, j * FOUT:(j + 1) * FOUT],
                                     AF.Exp, bias=negmax[0:jc, j:j + 1],
                                     accum_out=sumexp[0:jc, j:j + 1])
            nc.scalar.activation(lnsum[:], sumexp[:], AF.Ln)
            nc.vector.tensor_sub(adj[:], negmax[:], lnsum[:])
            for j in range(JC):
                jc = _jc(j)
                nc.vector.tensor_scalar_add(outsb[0:jc, j * FOUT:(j + 1) * FOUT],
                                            z2sb[0:jc, j * FOUT:(j + 1) * FOUT],
                                            adj[0:jc, j:j + 1])
            nfull = (JC - 1) * P  # 1152 nodes in full chunks
            nc.sync.dma_start(
                out_d[0:nfull, :].rearrange("(j p) f -> p j f", p=P),
                outsb[:, 0:(JC - 1) * FOUT])
            nc.sync.dma_start(out_d[nfull:SHARD, :],
                              outsb[0:_jc(JC - 1),
                                    (JC - 1) * FOUT:JC * FOUT])

    nc.compile()
    return nc


_NC = None


def _get_nc():
    global _NC
    if _NC is None:
        _NC = build()
    return _NC


def make_in_maps(inputs):
    x = np.asarray(inputs["x"], dtype=np.float32)
    ei = np.asarray(inputs["edge_index"])
    src = ei[0].astype(np.int64)
    dst = ei[1].astype(np.int64)

    cnt = np.bincount(dst, minlength=N_NODES).astype(np.float32)
    inv = (1.0 / np.maximum(cnt, 1.0)).astype(np.float32)

    # dense edge-count matrix with padded src slots, partition-major
    srcp = (src // SHARD) * SLOTS + (src % SHARD)
    flat = srcp * N_NODES + dst
    counts = np.bincount(flat, minlength=G * N_NODES)
    A = counts.reshape(G, N_NODES).astype(np.float32)
    del counts
    A = np.clip(A, 0.0, 240.0)
    A8 = A.astype(NP_FP8).reshape(KC, P, N_NODES).transpose(1, 0, 2)
    del A

    # padded transposed features [128, 10240]
    xp = np.zeros((G, F), np.float32)
    for c in range(N_CORES):
        xp[c * SLOTS:c * SLOTS + SHARD] = x[c * SHARD:(c + 1) * SHARD]
    xt = np.ascontiguousarray(xp.T).astype(NP_BF16)

    def w(name):
        return np.ascontiguousarray(
            np.asarray(inputs[name], np.float32)).astype(NP_BF16)

    common = {
        "xt": xt,
        "lin_w0": w("lin_w0"), "lin_w1": w("lin_w1"),
        "agg_w0": w("agg_w0"), "agg_w1": w("agg_w1"),
        "mp_w1": w("mp_w1"), "mp_w2": w("mp_w2"),
    }
    in_maps = []
    for c in range(N_CORES):
        lo, hi = c * SHARD, (c + 1) * SHARD
        in_maps.append({
            **common,
            "xt_sh": np.ascontiguousarray(xt[:, c * SLOTS:(c + 1) * SLOTS]),
            "a8": np.ascontiguousarray(A8[:, :, lo:hi]),
            "inv": np.ascontiguousarray(
                np.broadcast_to(inv[lo:hi][None, :], (P, SHARD))).astype(
                    np.float32),
        })
    return in_maps


def run(inputs, trace=False, **kwargs):
    nc = _get_nc()
    in_maps = make_in_maps(inputs)
    res = run_bass_kernel_spmd(nc, in_maps, core_ids=list(range(N_CORES)),
                               trace=trace, **kwargs)
    out = np.concatenate([res.results[c]["out"] for c in range(N_CORES)],
                         axis=0)
    return out.astype(np.float32), res


def kernel(**inputs):
    out, _ = run(inputs, trace=False)
    return out
